# revision 5
# baseline (speedup 1.0000x reference)
"""LOUPE sampler kernel for 8 Trainium2 NeuronCores.

reference semantics:
  prob_mask = sigmoid(5 * mask_logits)            # (320,)
  rescaled  = RescaleProbMap(prob_mask, sparsity)
  b         = binarize(rescaled)                  # rejection sampling, {0,1}
  b[0] = b[-1] = 1
  masked_kspace = b[None,None,:,None] * kspace    # (256,2,320,320)
  mask_full     = broadcast(b) -> (256,1,320,320)

The mask is batch-independent and binary, so the device work is pure data
movement: copy kspace lines where mask==1, write 1.0 rows of mask_full,
leave everything else at the pre-zeroed output value (run_bass_kernel_spmd
pre-zeros ExternalOutput buffers; kernels that don't write every element
rely on that).  Sharding: data-parallel over batch, 32 per core.
"""

import os
import sys

import numpy as np

try:
    import concourse.bass as bass  # noqa: F401
except ImportError:  # fresh grading dir: make the repo importable
    sys.path.insert(0, "/opt/trn_rl_repo")

L = 320
W = 320
N_CORES = 8
B = 256
B_LOC = B // N_CORES            # 32 batches per core
SLABS = B_LOC * 2               # 64 (batch, channel) slabs per core
SLOPE = 5.0
MAX_RUN = 40                    # split longer keep/zero runs into <= MAX_RUN lines

# Set KERNEL_WRITE_ZEROS=1 to explicitly write the zero regions instead of
# relying on pre-zeroed outputs.
WRITE_ZEROS = os.environ.get("KERNEL_WRITE_ZEROS", "0") == "1"
TRACE = os.environ.get("KERNEL_TRACE", "0") == "1"

_last_results = None  # BassKernelResults of the most recent kernel() call


def _compute_mask(mask_logits: np.ndarray, sparsity: np.ndarray) -> np.ndarray:
    """Exact replication of the reference mask pipeline on jax-CPU.

    threefry PRNG is backend-deterministic, so running on CPU reproduces the
    reference bit-for-bit regardless of where the oracle ran.
    """
    import jax
    import jax.numpy as jnp

    cpu = jax.devices("cpu")[0]
    with jax.default_device(cpu):
        logits = jnp.asarray(np.asarray(mask_logits), jnp.float32)
        sp = jnp.asarray(np.asarray(sparsity), jnp.float32)
        x = jax.nn.sigmoid(SLOPE * logits).reshape(1, 1, L, 1)
        # RescaleProbMap
        xbar = jnp.mean(x)
        r = sp / xbar
        beta = (1.0 - sp) / (1.0 - xbar)
        le = (r <= 1.0).astype(x.dtype)
        x = le * x * r + (1.0 - le) * (1.0 - (1.0 - x) * beta)
        # binarize: rejection-sample uniforms until mean matches
        atol, rtol = 1e-3, 1e-5
        xm = jnp.mean(x)
        key = jax.random.key(42)
        while True:
            key, sub = jax.random.split(key)
            prob = jax.random.uniform(sub, x.shape, x.dtype)
            result = (x > prob).astype(x.dtype)
            if bool(jnp.abs(jnp.mean(result) - xm) <= atol + rtol * jnp.abs(xm)):
                break
        b = result.at[..., :1, :].set(1.0).at[..., -1:, :].set(1.0)
        return np.asarray(b, np.float32).reshape(L)


def _runs(flags) -> list[tuple[int, int]]:
    """Maximal runs of consecutive True lines, split to length <= MAX_RUN."""
    out = []
    i = 0
    while i < L:
        if flags[i]:
            j = i
            while j < L and flags[j]:
                j += 1
            s = i
            while s < j:
                ln = min(MAX_RUN, j - s)
                out.append((s, ln))
                s += ln
            i = j
        else:
            i += 1
    return out


def _build_nc(keep_runs, zero_runs, write_zeros: bool):
    """Pure-DMA program: every instruction is an independent DRAM->DRAM copy
    (PSEUDO_DMA_DIRECT2D allows only one sync-wait, so no SBUF/compute
    producers anywhere -- constants come in as DRAM inputs)."""
    import concourse.bass as bass
    import concourse.mybir as mybir

    f32 = mybir.dt.float32
    nc = bass.Bass()
    ksp = nc.dram_tensor("ksp", (SLABS, L, W), f32, kind="ExternalInput")
    ones = nc.dram_tensor("ones", (B_LOC, MAX_RUN, W), f32, kind="ExternalInput")
    if write_zeros:
        zeros = nc.dram_tensor(
            "zeros", (SLABS, MAX_RUN, W), f32, kind="ExternalInput"
        )
    masked = nc.dram_tensor("masked", (SLABS, L, W), f32, kind="ExternalOutput")
    mfull = nc.dram_tensor("mfull", (B_LOC, L, W), f32, kind="ExternalOutput")

    # (out_ap, in_ap) per DMA, round-robined over the two HWDGE engines
    xfers = []
    for h0, ln in keep_runs:
        xfers.append((masked[:, h0 : h0 + ln, :], ksp[:, h0 : h0 + ln, :]))
    for h0, ln in keep_runs:
        xfers.append((mfull[:, h0 : h0 + ln, :], ones[:, :ln, :]))
    if write_zeros:
        for h0, ln in zero_runs:
            xfers.append((masked[:, h0 : h0 + ln, :], zeros[:, :ln, :]))
            xfers.append((mfull[:, h0 : h0 + ln, :], zeros[:B_LOC, :ln, :]))

    total = 16 * len(xfers)
    with nc.Block() as block, nc.semaphore("dma_sem") as dma_sem:

        @block.sync
        def _(eng: bass.BassEngine):
            for out_ap, in_ap in xfers[0::2]:
                eng.dma_start(out=out_ap, in_=in_ap).then_inc(dma_sem, 16)
            eng.wait_ge(dma_sem, total)

        @block.scalar
        def _(eng: bass.BassEngine):
            for out_ap, in_ap in xfers[1::2]:
                eng.dma_start(out=out_ap, in_=in_ap).then_inc(dma_sem, 16)
            eng.wait_ge(dma_sem, total)

    return nc


def kernel(kspace, sparsity, mask_logits):
    global _last_results
    from concourse.bass_utils import run_bass_kernel_spmd

    kspace = np.ascontiguousarray(np.asarray(kspace, np.float32))
    mask = _compute_mask(mask_logits, sparsity)
    keep = mask != 0.0
    keep_runs = _runs(keep)
    zero_runs = _runs(~keep)

    nc = _build_nc(keep_runs, zero_runs, WRITE_ZEROS)

    core_ids = list(range(N_CORES))
    ones_np = np.ones((B_LOC, MAX_RUN, W), np.float32)
    zeros_np = np.zeros((SLABS, MAX_RUN, W), np.float32)
    in_maps = []
    for c in core_ids:
        m = {
            "ksp": kspace[c * B_LOC : (c + 1) * B_LOC].reshape(SLABS, L, W),
            "ones": ones_np,
        }
        if WRITE_ZEROS:
            m["zeros"] = zeros_np
        in_maps.append(m)
    res = run_bass_kernel_spmd(nc, in_maps, core_ids, trace=TRACE)
    _last_results = res

    masked_kspace = np.empty((B, 2, L, W), np.float32)
    mask_full = np.empty((B, 1, L, W), np.float32)
    for c in core_ids:
        masked_kspace[c * B_LOC : (c + 1) * B_LOC] = res.results[c]["masked"].reshape(
            B_LOC, 2, L, W
        )
        mask_full[c * B_LOC : (c + 1) * B_LOC] = res.results[c]["mfull"].reshape(
            B_LOC, 1, L, W
        )
    return masked_kspace, mask_full
